# revision 1
# baseline (speedup 1.0000x reference)
"""Multi-head attention (N=2, L=2048, 16 heads x 64) on 8 TRN2 NeuronCores.

Sharding: head-parallel attention (2 heads/core, both batches), then one
8-core AllToAll to switch to sequence-parallel for the output projection.
All matmuls bf16 with fp32 accumulation; softmax in fp32 (exp on ScalarE,
denominator via a ones-column appended to V).

Orientation: scores are computed transposed ([k, q]) so the attention
weights feed the AV matmul as the moving operand (stationary = V tiles,
full 128-row weights -> fast-weight-load). The AV output [dv+1, q] lands
with the denominator in row 64; normalization is 1/den broadcast via a
rank-1 PE matmul and applied on VectorE. The output projection consumes
the AllToAll result directly and produces the final output transposed;
the host transposes back.
"""
import sys

sys.path.insert(0, "/opt/trn_rl_repo")

import numpy as np
import ml_dtypes

import concourse.bass as bass
import concourse.bacc as bacc
import concourse.mybir as mybir
import concourse.tile as tile
from concourse.bass_utils import run_bass_kernel_spmd

BF16 = ml_dtypes.bfloat16

DM = 1024      # dmodel
DK = 64        # head dim
H = 16         # heads
NB = 2         # batch
L = 2048       # seq len
R = NB * L     # combined rows
NC = 8         # cores
HPC = H // NC  # heads per core = 2
DPC = HPC * DK  # depth per core = 128

SW = 512       # sub-window (AV, masks, output chunk)
WW = 1024      # score/exp window
KT = 128       # k tile
NSW = L // SW   # 4 sub-windows per batch
NWW = L // WW   # 2 score windows per batch
NKT = L // KT   # 16 k tiles per batch
CHUNK = R // NC  # 512 combined rows per core

_CACHE = {}


def _classify_blocks(mask):
    """Per 512-granular (qs, kt): 0=skip, 1=full, 2=partial (+ q-span, pattern)."""
    mask = np.asarray(mask, dtype=bool)
    cls = [[0] * NKT for _ in range(NSW)]
    span = [[None] * NKT for _ in range(NSW)]
    pat_ids = {}
    pats = []
    pat_idx = [[-1] * NKT for _ in range(NSW)]
    for qs in range(NSW):
        for kt in range(NKT):
            sub = mask[qs * SW:(qs + 1) * SW, kt * KT:(kt + 1) * KT]
            rows = np.nonzero(sub.any(axis=1))[0]
            if rows.size == 0:
                cls[qs][kt] = 0
            elif sub.all():
                cls[qs][kt] = 1
                span[qs][kt] = (0, SW)
            else:
                cls[qs][kt] = 2
                span[qs][kt] = (int(rows[0]), int(rows[-1]) + 1)
                pat = np.ascontiguousarray(sub.T).astype(BF16)  # [128 k, SW q]
                key = pat.tobytes()
                if key not in pat_ids:
                    pat_ids[key] = len(pats)
                    pats.append(pat)
                pat_idx[qs][kt] = pat_ids[key]
    # general-mask safety: the first included kt of each sub-window must cover
    # the full 512 columns (its start=True matmul clears PSUM has_written)
    for qs in range(NSW):
        for kt in range(NKT):
            if cls[qs][kt]:
                span[qs][kt] = (0, SW)
                break
    if not pats:
        pats.append(np.ones((KT, SW), dtype=BF16))
    return cls, span, pat_idx, np.stack(pats)


def _build(cls, span, pat_idx, n_pat):
    nc = bacc.Bacc("TRN2", target_bir_lowering=False, debug=False,
                   enable_asserts=False, num_devices=NC)
    f32, bf16 = mybir.dt.float32, mybir.dt.bfloat16

    xtb = nc.dram_tensor("xtb", [DM, R], bf16, kind="ExternalInput")
    ytb = nc.dram_tensor("ytb", [DM, R], bf16, kind="ExternalInput")
    wq = nc.dram_tensor("wq", [DM, DPC], bf16, kind="ExternalInput")
    wk = nc.dram_tensor("wk", [DM, DPC], bf16, kind="ExternalInput")
    wv = nc.dram_tensor("wv", [DM, HPC * 65], bf16, kind="ExternalInput")
    wo = nc.dram_tensor("wo", [DM, DM], bf16, kind="ExternalInput")
    bqd = nc.dram_tensor("bq", [DPC, 1], f32, kind="ExternalInput")
    bkd = nc.dram_tensor("bk", [DPC, 1], f32, kind="ExternalInput")
    bv1 = nc.dram_tensor("bv1", [1, HPC * 65], bf16, kind="ExternalInput")
    bod = nc.dram_tensor("bo", [DM, 1], f32, kind="ExternalInput")
    mpat = nc.dram_tensor("mpat", [n_pat, KT, SW], bf16, kind="ExternalInput")
    out_t = nc.dram_tensor("out_t", [DM, CHUNK], f32, kind="ExternalOutput")

    VW = 65 * HPC  # v_aug width per k-tile (both heads)

    # per score-window (qw within batch): which kt are included, and the
    # union span of valid q columns in window coordinates
    def window_kts(qw):
        out = []
        for kt in range(NKT):
            lo, hi = None, None
            for s in range(WW // SW):
                qs = qw * (WW // SW) + s
                if cls[qs][kt]:
                    a, b = span[qs][kt]
                    a += s * SW
                    b += s * SW
                    lo = a if lo is None else min(lo, a)
                    hi = b if hi is None else max(hi, b)
            if lo is not None:
                out.append((kt, lo, hi))
        return out

    with tile.TileContext(nc) as tc:
        with (
            tc.tile_pool(name="const", bufs=1) as cst,
            tc.tile_pool(name="xy", bufs=12) as xy,
            tc.tile_pool(name="big", bufs=1) as big,
            tc.tile_pool(name="exp", bufs=14) as expp,
            tc.tile_pool(name="sm", bufs=4) as sm,
            tc.tile_pool(name="osb", bufs=3) as osb,
            tc.tile_pool(name="sp", bufs=2, space="PSUM") as sp,
            tc.tile_pool(name="avp", bufs=3, space="PSUM") as avp,
            tc.tile_pool(name="dram", bufs=1, space="DRAM") as dram,
            tc.tile_pool(name="dscr", bufs=4, space="DRAM") as dscrp,
        ):
            # ---- constants to SBUF ----
            wq_sb = cst.tile([128, 8 * DPC], bf16)
            wk_sb = cst.tile([128, 8 * DPC], bf16)
            wv_sb = cst.tile([128, 8 * VW], bf16)
            wo_sb = cst.tile([128, 8 * DM], bf16)
            for dt in range(8):
                nc.sync.dma_start(wq_sb[:, dt * DPC:(dt + 1) * DPC], wq[dt * 128:(dt + 1) * 128, :])
                nc.sync.dma_start(wk_sb[:, dt * DPC:(dt + 1) * DPC], wk[dt * 128:(dt + 1) * 128, :])
                nc.sync.dma_start(wv_sb[:, dt * VW:(dt + 1) * VW], wv[dt * 128:(dt + 1) * 128, :])
                nc.sync.dma_start(wo_sb[:, dt * DM:(dt + 1) * DM], wo[dt * 128:(dt + 1) * 128, :])
            bq_sb = cst.tile([DPC, 1], f32)
            bk_sb = cst.tile([DPC, 1], f32)
            nc.sync.dma_start(bq_sb[:], bqd[:])
            nc.sync.dma_start(bk_sb[:], bkd[:])
            bv1_sb = cst.tile([1, VW], bf16)
            nc.sync.dma_start(bv1_sb[:], bv1[:])
            bo_sb = cst.tile([128, 8], f32)
            for mt in range(8):
                nc.sync.dma_start(bo_sb[:, mt:mt + 1], bod[mt * 128:(mt + 1) * 128, :])
            mpat_sb = cst.tile([KT, n_pat * SW], bf16)
            for p in range(n_pat):
                nc.sync.dma_start(mpat_sb[:, p * SW:(p + 1) * SW], mpat[p])
            ones_row = cst.tile([1, 128], bf16)
            nc.vector.memset(ones_row[:], 1.0)

            # start-of-kernel barrier: absorbs per-core launch skew while the
            # big input DMAs stream, so the real AllToAll later isn't skewed
            bar_in = dram.tile([1, 8], f32)
            bar_out = dram.tile([1, 8], f32)
            barrier_sb = cst.tile([1, 8], f32, tag="barrier_sb")
            nc.vector.memset(barrier_sb[:], 0.0)
            nc.sync.dma_start(bar_in[:], barrier_sb[:])
            nc.gpsimd.collective_compute(
                "AllReduce", mybir.AluOpType.add,
                replica_groups=[list(range(NC))],
                ins=[bar_in.opt()], outs=[bar_out.opt()])

            qT = big.tile([DPC, R], bf16)
            kT = big.tile([DPC, R], bf16)
            vaug = big.tile([128, (R // KT) * VW], bf16)
            headT0 = big.tile([DK, R], bf16)
            headT1 = big.tile([DK, R], bf16)
            headT = [headT0, headT1]

            # ---- projections ----
            for src, wsb, bias, dst, do_v in ((xtb, wq_sb, bq_sb, qT, False),
                                              (ytb, wk_sb, bk_sb, kT, True)):
                for qc in range(R // 1024):
                    tiles = []
                    for dt in range(8):
                        t = xy.tile([128, 1024], bf16, tag="xy")
                        nc.sync.dma_start(t[:], src[dt * 128:(dt + 1) * 128,
                                                    qc * 1024:(qc + 1) * 1024])
                        tiles.append(t)
                    for s in range(2):
                        ps = sp.tile([128, WW], f32, tag="sp")
                        for dt in range(8):
                            nc.tensor.matmul(ps[:DPC, :SW], wsb[:, dt * DPC:(dt + 1) * DPC],
                                             tiles[dt][:, s * SW:(s + 1) * SW],
                                             start=(dt == 0), stop=(dt == 7))
                        col = qc * 1024 + s * SW
                        nc.scalar.activation(dst[:, col:col + SW], ps[:DPC, :SW],
                                             mybir.ActivationFunctionType.Identity,
                                             bias=bias[:])
                    if do_v:
                        for j in range(8):
                            kti = qc * 8 + j
                            psv = sp.tile([128, WW], f32, tag="sp")
                            for dt in range(8):
                                nc.tensor.matmul(psv[:, :VW],
                                                 tiles[dt][:, j * KT:(j + 1) * KT],
                                                 wv_sb[:, dt * VW:(dt + 1) * VW],
                                                 start=(dt == 0), stop=False)
                            nc.tensor.matmul(psv[:, :VW], ones_row[:],
                                             bv1_sb[:], start=False, stop=True)
                            nc.scalar.copy(vaug[:, kti * VW:kti * VW + VW],
                                           psv[:, :VW])

            # ---- attention ----
            a2a_in = dram.tile([NC, DPC, CHUNK], bf16)
            a2a_out = dram.tile([NC, DPC, CHUNK], bf16)
            worder = [(n, qw) for n in range(NB) for qw in range(NWW)]
            for n, qw in worder:
                    wkts = window_kts(qw)
                    wcol = n * L + qw * WW  # window base in combined cols
                    exp_tiles = {}
                    for kt, lo, hi in wkts:
                        # both heads' score matmuls adjacent chunk-major:
                        # T0/T8 row tiles run concurrently on the PE
                        pss = []
                        for hp in range(HPC):
                            hs = hp * DK
                            ps = sp.tile([128, WW], f32, tag="sp", name=f"ps{hp}")
                            for a, b in ((lo, min(hi, SW)), (max(lo, SW), hi)):
                                if a >= b:
                                    continue
                                nc.tensor.matmul(
                                    ps[:KT, a:b],
                                    kT[hs:hs + DK, n * L + kt * KT:n * L + (kt + 1) * KT],
                                    qT[hs:hs + DK, wcol + a:wcol + b],
                                    start=True, stop=True)
                            pss.append(ps)
                        for hp in range(HPC):
                            et = expp.tile([KT, WW], bf16, tag="exp")
                            nc.scalar.activation(et[:, lo:hi], pss[hp][:KT, lo:hi],
                                                 mybir.ActivationFunctionType.Exp)
                            exp_tiles[kt, hp] = et
                    for s in range(WW // SW):
                        qs = qw * (WW // SW) + s
                        kts = [kt for (kt, h) in exp_tiles if cls[qs][kt] and h == 0]
                        if not kts:
                            continue
                        for hp in range(HPC):
                            # mask partial blocks (on the exp values)
                            for kt in kts:
                                if cls[qs][kt] == 2:
                                    a, b = span[qs][kt]
                                    p = pat_idx[qs][kt]
                                    nc.vector.tensor_tensor(
                                        exp_tiles[kt, hp][:, s * SW + a:s * SW + b],
                                        exp_tiles[kt, hp][:, s * SW + a:s * SW + b],
                                        mpat_sb[:, p * SW + a:p * SW + b],
                                        mybir.AluOpType.mult)
                            av = avp.tile([65, SW], f32, tag="avp")
                            for i, kt in enumerate(kts):
                                a, b = span[qs][kt]
                                nc.tensor.matmul(
                                    av[:, a:b],
                                    vaug[:, (n * NKT + kt) * VW + hp * 65:
                                         (n * NKT + kt) * VW + (hp + 1) * 65],
                                    exp_tiles[kt, hp][:, s * SW + a:s * SW + b],
                                    start=(i == 0), stop=(i == len(kts) - 1))
                            # normalization: reshape den row across lanes,
                            # exact reciprocal, broadcast via stride-0 DMA
                            den = sm.tile([65, SW], f32, tag="den")
                            nc.vector.tensor_copy(den[64:65, :], av[64:65, :])
                            d128 = sm.tile([128, SW // 128], f32, tag="d128")
                            nc.sync.dma_start(d128[:], den[64:65, :])
                            r128 = sm.tile([128, SW // 128], f32, tag="r128")
                            nc.vector.reciprocal(r128[:], d128[:])
                            dsc = dscrp.tile([1, SW], f32, tag="dscr")
                            nc.sync.dma_start(dsc[:], r128[:])
                            bcs = sm.tile([DK, SW], f32, tag="bcs")
                            nc.sync.dma_start(bcs[:], dsc[:].to_broadcast([DK, SW]))
                            ccol = n * L + qs * SW
                            nc.vector.tensor_tensor(
                                headT[hp][:, ccol:ccol + SW], av[:DK, :], bcs[:],
                                mybir.AluOpType.mult)
                    # both heads done for this window: ship its two shards
                    for s in range(WW // SW):
                        r = (n * L + qw * WW + s * SW) // CHUNK
                        for hp in range(HPC):
                            nc.sync.dma_start(
                                a2a_in[r][hp * DK:(hp + 1) * DK, :],
                                headT[hp][:, r * CHUNK:(r + 1) * CHUNK])

            # ---- AllToAll: head-split -> sequence-split ----
            nc.gpsimd.collective_compute(
                "AllToAll", mybir.AluOpType.bypass,
                replica_groups=[list(range(NC))],
                ins=[a2a_in.opt()], outs=[a2a_out.opt()])

            # keep the PE clock warm during the collective wait so the output
            # projection runs at full rate (gated on the LAST-written headT
            # columns = end of attention)
            for i in range(24):
                wps = sp.tile([128, WW], f32, tag="sp", name=f"warm{i}")
                nc.tensor.matmul(wps[:, :SW], headT0[:, R - 128:R],
                                 headT0[:, R - SW:R],
                                 start=True, stop=True)

            # ---- output projection (result transposed: [dmodel, chunk]) ----
            rhs = []
            for jj in range(8):
                t = expp.tile([DPC, CHUNK], bf16, tag="exp")
                nc.sync.dma_start(t[:], a2a_out[jj])
                rhs.append(t)
            for mt in range(8):
                ps = sp.tile([128, WW], f32, tag="sp")
                for jj in range(8):
                    nc.tensor.matmul(ps[:, :SW],
                                     wo_sb[:, jj * DM + mt * 128:jj * DM + (mt + 1) * 128],
                                     rhs[jj][:], start=(jj == 0), stop=(jj == 7))
                ob = osb.tile([128, CHUNK], f32, tag="osb")
                nc.vector.tensor_scalar_add(ob[:], ps[:, :SW], bo_sb[:, mt:mt + 1])
                nc.sync.dma_start(out_t[mt * 128:(mt + 1) * 128, :], ob[:])

    nc.compile()
    return nc


def kernel(x, y, mask, Wq, bq, Wk, bk, Wv, bv, Wo, bo, _trace=False):
    x = np.asarray(x, np.float32)
    y = np.asarray(y, np.float32)
    cls, span, pat_idx, pats = _classify_blocks(mask)

    key = (x.shape,
           tuple(tuple(c) for c in cls),
           tuple(tuple(s) for s in span),
           tuple(tuple(p) for p in pat_idx),
           pats.tobytes())
    if key not in _CACHE:
        _CACHE[key] = _build(cls, span, pat_idx, pats.shape[0])
    nc = _CACHE[key]

    fac = np.float32(1.0 / np.sqrt(DK))
    xtb = np.ascontiguousarray(
        np.concatenate([x[n].T for n in range(NB)], axis=1)).astype(BF16)
    ytb = np.ascontiguousarray(
        np.concatenate([y[n].T for n in range(NB)], axis=1)).astype(BF16)
    Wq32 = np.asarray(Wq, np.float32) * fac
    bq32 = np.asarray(bq, np.float32) * fac

    in_maps = []
    for c in range(NC):
        d0 = c * DPC
        wv_aug = np.zeros((DM, HPC * 65), np.float32)
        bv1 = np.zeros((1, HPC * 65), np.float32)
        for hp in range(HPC):
            h = HPC * c + hp
            wv_aug[:, hp * 65:hp * 65 + DK] = np.asarray(Wv, np.float32)[:, h * DK:(h + 1) * DK]
            bv1[0, hp * 65:hp * 65 + DK] = np.asarray(bv, np.float32)[h * DK:(h + 1) * DK]
            bv1[0, hp * 65 + DK] = 1.0
        in_maps.append({
            "xtb": xtb, "ytb": ytb,
            "wq": Wq32[:, d0:d0 + DPC].astype(BF16),
            "wk": np.asarray(Wk, np.float32)[:, d0:d0 + DPC].astype(BF16),
            "wv": wv_aug.astype(BF16),
            "wo": np.asarray(Wo, np.float32).astype(BF16),
            "bq": bq32[d0:d0 + DPC].reshape(DPC, 1),
            "bk": np.asarray(bk, np.float32)[d0:d0 + DPC].reshape(DPC, 1),
            "bv1": bv1.astype(BF16),
            "bo": np.asarray(bo, np.float32).reshape(DM, 1),
            "mpat": pats,
        })

    res = run_bass_kernel_spmd(nc, in_maps, core_ids=list(range(NC)), trace=_trace)
    out = np.empty((NB, L, DM), np.float32)
    for c in range(NC):
        n = c // (NC // NB)
        q0 = CHUNK * (c % (NC // NB))
        out[n, q0:q0 + CHUNK, :] = res.results[c]["out_t"].T
    if _trace:
        kernel.last_results = res
    return out



# revision 4
# speedup vs baseline: 1.0923x; 1.0923x over previous
"""Multi-head attention (N=2, L=2048, 16 heads x 64) on 8 TRN2 NeuronCores.

v2: head-parallel attention (2 heads/core) with a per-batch software pipeline:
load+project batch 0 -> attention batch 0 (overlapped with batch-1 loads and
projections) -> per-batch AllToAll (head-split -> sequence-split), each
overlapped with the next phase -> output projection per batch.

Key scheduling choices vs v1:
- Few, large DMAs with 3D access patterns (one per weight tensor, one per
  input quarter), split across the two HWDGE rings (sync + scalar).
- Softmax normalization without DMA: denominator row -> DVE reciprocal ->
  PE rank-1 broadcast matmul -> DVE multiply.
- Scores/exp/AV processed per 512-q sub-window; exp is one ScalarE call per
  k-tile covering both heads via a 3D access pattern.
- AllToAll split per batch so collective #0 hides under batch-1 attention
  and the batch-0 output projection hides under collective #1.
"""
import sys

sys.path.insert(0, "/opt/trn_rl_repo")

import numpy as np
import ml_dtypes

import concourse.bass as bass
import concourse.bacc as bacc
import concourse.mybir as mybir
import concourse.tile as tile
from concourse.bass_utils import run_bass_kernel_spmd

BF16 = ml_dtypes.bfloat16

DM = 1024      # dmodel
DK = 64        # head dim
H = 16         # heads
NB = 2         # batch
L = 2048       # seq len
R = NB * L
NC = 8         # cores
HPC = H // NC  # heads per core = 2
DPC = HPC * DK  # depth per core = 128

SW = 512       # q sub-window
KT = 128       # k tile
NQS = L // SW   # 4 q blocks per batch
NKT = L // KT   # 16 k tiles per batch
CB = L // NC    # 256: per-batch per-core output chunk
VW = 65 * HPC   # 130: augmented v width (both heads, +ones col each)

_CACHE = {}


def _classify_blocks(mask):
    """Per (qs, kt) block: 0=skip, 1=full, 2=partial (+ q-span, pattern)."""
    mask = np.asarray(mask, dtype=bool)
    cls = [[0] * NKT for _ in range(NQS)]
    span = [[None] * NKT for _ in range(NQS)]
    pat_ids = {}
    pats = []
    pat_idx = [[-1] * NKT for _ in range(NQS)]
    for qs in range(NQS):
        for kt in range(NKT):
            sub = mask[qs * SW:(qs + 1) * SW, kt * KT:(kt + 1) * KT]
            rows = np.nonzero(sub.any(axis=1))[0]
            if rows.size == 0:
                cls[qs][kt] = 0
            elif sub.all():
                cls[qs][kt] = 1
                span[qs][kt] = (0, SW)
            else:
                cls[qs][kt] = 2
                span[qs][kt] = (int(rows[0]), int(rows[-1]) + 1)
                pat = np.ascontiguousarray(sub.T).astype(BF16)  # [128 k, SW q]
                key = pat.tobytes()
                if key not in pat_ids:
                    pat_ids[key] = len(pats)
                    pats.append(pat)
                pat_idx[qs][kt] = pat_ids[key]
    # the first included kt of each sub-window must cover the full 512
    # columns (its start=True matmul clears PSUM has_written)
    for qs in range(NQS):
        for kt in range(NKT):
            if cls[qs][kt]:
                span[qs][kt] = (0, SW)
                break
    if not pats:
        pats.append(np.ones((KT, SW), dtype=BF16))
    return cls, span, pat_idx, np.stack(pats)


def _build(cls_, span_, pidx, n_pat):
    nc = bacc.Bacc("TRN2", target_bir_lowering=False, debug=False,
                   enable_asserts=False, num_devices=NC)
    f32, bf16 = mybir.dt.float32, mybir.dt.bfloat16
    EXP = mybir.ActivationFunctionType.Exp
    MUL = mybir.AluOpType.mult

    xtb = nc.dram_tensor("xtb", [DM, R], bf16, kind="ExternalInput")
    ytb = nc.dram_tensor("ytb", [DM, R], bf16, kind="ExternalInput")
    wq = nc.dram_tensor("wq", [DM, DPC], bf16, kind="ExternalInput")
    wk = nc.dram_tensor("wk", [DM, DPC], bf16, kind="ExternalInput")
    wv = nc.dram_tensor("wv", [DM, VW], bf16, kind="ExternalInput")
    wo = nc.dram_tensor("wo", [DM, DM], bf16, kind="ExternalInput")
    bqd = nc.dram_tensor("bq", [DPC, 1], f32, kind="ExternalInput")
    bkd = nc.dram_tensor("bk", [DPC, 1], f32, kind="ExternalInput")
    bv1 = nc.dram_tensor("bv1", [1, VW], bf16, kind="ExternalInput")
    bod = nc.dram_tensor("bo", [DM, 1], f32, kind="ExternalInput")
    mpat = nc.dram_tensor("mpat", [n_pat, KT, SW], bf16, kind="ExternalInput")
    out_t = nc.dram_tensor("out_t", [DM, NB * CB], f32, kind="ExternalOutput")

    with tile.TileContext(nc) as tc:
        with (
            tc.tile_pool(name="cst", bufs=1) as cst,
            tc.tile_pool(name="xy", bufs=5) as xy,
            tc.tile_pool(name="big", bufs=1) as big,
            tc.tile_pool(name="expp", bufs=24) as expp,
            tc.tile_pool(name="nrm", bufs=3) as nrm,
            tc.tile_pool(name="wos", bufs=2) as wos,
            tc.tile_pool(name="osb", bufs=3) as osb,
            tc.tile_pool(name="sp", bufs=2, space="PSUM") as sp,
            tc.tile_pool(name="avp", bufs=2, space="PSUM") as avp,
            tc.tile_pool(name="bcp", bufs=2, space="PSUM") as bcp,
            tc.tile_pool(name="dram", bufs=1, space="DRAM") as dram,
        ):
            # ---- constants (scalar HWDGE ring; one DMA per tensor) ----
            wq_sb = cst.tile([128, 8, DPC], bf16)
            wk_sb = cst.tile([128, 8, DPC], bf16)
            wv_sb = cst.tile([128, 8, VW], bf16)
            wo_sb = cst.tile([128, 8, DM], bf16)
            nc.scalar.dma_start(wq_sb[:], wq[:, :].rearrange("(t p) d -> p t d", p=128))
            nc.scalar.dma_start(wk_sb[:], wk[:, :].rearrange("(t p) d -> p t d", p=128))
            nc.scalar.dma_start(wv_sb[:], wv[:, :].rearrange("(t p) d -> p t d", p=128))
            bq_sb = cst.tile([DPC, 1], f32)
            bk_sb = cst.tile([DPC, 1], f32)
            bv1_sb = cst.tile([1, VW], bf16)
            bo_sb = cst.tile([128, 8, 1], f32)
            nc.scalar.dma_start(bq_sb[:], bqd[:])
            nc.scalar.dma_start(bk_sb[:], bkd[:])
            nc.scalar.dma_start(bv1_sb[:], bv1[:])
            nc.scalar.dma_start(bo_sb[:], bod[:, :].rearrange("(t p) o -> p t o", p=128))
            mpat_sb = cst.tile([KT, n_pat, SW], bf16)
            nc.scalar.dma_start(mpat_sb[:], mpat[:, :, :].transpose([1, 0, 2]))
            nc.scalar.dma_start(wo_sb[:], wo[:, :].rearrange("(t p) m -> p t m", p=128))
            ones_row = cst.tile([1, 128], bf16)
            nc.vector.memset(ones_row[:], 1.0)
            ones65 = cst.tile([65, DK], bf16)
            nc.vector.memset(ones65[:], 1.0)

            # preload the exp table set during the DMA phase
            bar_sb = cst.tile([1, 8], f32)
            nc.vector.memset(bar_sb[:], 0.0)
            dum = cst.tile([1, 8], f32)
            nc.scalar.activation(dum[:], bar_sb[:], EXP)

            # ---- start-of-kernel barrier (absorbs launch skew) ----
            bar_in = dram.tile([1, 8], f32, tag="bar_in")
            bar_out = dram.tile([1, 8], f32, tag="bar_out")
            nc.gpsimd.dma_start(bar_in[:], bar_sb[:])
            nc.gpsimd.collective_compute(
                "AllReduce", mybir.AluOpType.add,
                replica_groups=[list(range(NC))],
                ins=[bar_in.opt()], outs=[bar_out.opt()])

            qT = [big.tile([DPC, L], bf16, tag=f"qT{n}", name=f"qT{n}") for n in range(NB)]
            kT = [big.tile([DPC, L], bf16, tag=f"kT{n}", name=f"kT{n}") for n in range(NB)]
            vaug = [big.tile([128, NKT * VW], bf16, tag=f"va{n}", name=f"va{n}") for n in range(NB)]
            headT = [[big.tile([DK, L], bf16, tag=f"hT{n}{hp}", name=f"hT{n}{hp}")
                      for hp in range(HPC)] for n in range(NB)]

            a2a_in = [dram.tile([NC, DPC, CB], bf16, tag=f"a2ai{n}", name=f"a2ai{n}")
                      for n in range(NB)]
            a2a_out = [dram.tile([NC, DPC, CB], bf16, tag=f"a2ao{n}", name=f"a2ao{n}")
                       for n in range(NB)]

            ytile = [[None] * NQS for _ in range(NB)]
            xtile = [[None] * NQS for _ in range(NB)]

            def emit_inputs(n):
                for b in range(NQS):
                    yt = xy.tile([128, 8, SW], bf16, tag="xy", name=f"y{n}b{b}")
                    nc.sync.dma_start(
                        yt[:], ytb[:, n * L + b * SW:n * L + (b + 1) * SW]
                        .rearrange("(t p) c -> p t c", p=128))
                    ytile[n][b] = yt
                    xt = xy.tile([128, 8, SW], bf16, tag="xy", name=f"x{n}b{b}")
                    nc.sync.dma_start(
                        xt[:], xtb[:, n * L + b * SW:n * L + (b + 1) * SW]
                        .rearrange("(t p) c -> p t c", p=128))
                    xtile[n][b] = xt

            def emit_proj_block(n, b):
                yt = ytile[n][b]
                ps = sp.tile([128, 1024], f32, tag="sp", name=f"kp{n}{b}")
                for dt in range(8):
                    nc.tensor.matmul(ps[:DPC, :SW], wk_sb[:, dt, :], yt[:, dt, :],
                                     start=(dt == 0), stop=(dt == 7))
                nc.vector.tensor_scalar_add(kT[n][:, b * SW:(b + 1) * SW],
                                            ps[:DPC, :SW], bk_sb[:])
                for j in range(4):
                    kti = b * 4 + j
                    psv = sp.tile([128, 1024], f32, tag="sp", name=f"vp{n}{kti}")
                    for dt in range(8):
                        nc.tensor.matmul(psv[:, :VW], yt[:, dt, j * KT:(j + 1) * KT],
                                         wv_sb[:, dt, :], start=(dt == 0), stop=False)
                    nc.tensor.matmul(psv[:, :VW], ones_row[:], bv1_sb[:],
                                     start=False, stop=True)
                    nc.vector.tensor_copy(vaug[n][:, kti * VW:(kti + 1) * VW],
                                          psv[:, :VW])
                xt = xtile[n][b]
                ps2 = sp.tile([128, 1024], f32, tag="sp", name=f"qp{n}{b}")
                for dt in range(8):
                    nc.tensor.matmul(ps2[:DPC, :SW], wq_sb[:, dt, :], xt[:, dt, :],
                                     start=(dt == 0), stop=(dt == 7))
                nc.vector.tensor_scalar_add(qT[n][:, b * SW:(b + 1) * SW],
                                            ps2[:DPC, :SW], bq_sb[:])

            def emit_attn_qs(n, qs):
                kts = [kt for kt in range(NKT) if cls_[qs][kt]]
                exp_tiles = {}
                for kt in kts:
                    a, b = span_[qs][kt]
                    ps = sp.tile([128, 1024], f32, tag="sp", name=f"s{n}{qs}{kt}")
                    for hp in range(HPC):
                        hs = hp * DK
                        nc.tensor.matmul(
                            ps[:KT, hp * SW + a:hp * SW + b],
                            kT[n][hs:hs + DK, kt * KT:(kt + 1) * KT],
                            qT[n][hs:hs + DK, qs * SW + a:qs * SW + b],
                            start=True, stop=True)
                    et = expp.tile([128, 2, SW], bf16, tag="exp", name=f"e{n}{qs}{kt}")
                    nc.scalar.activation(
                        et[:, :, a:b],
                        ps.rearrange("p (h c) -> p h c", h=2)[:, :, a:b], EXP)
                    if cls_[qs][kt] == 2:
                        pi = pidx[qs][kt]
                        for hp in range(HPC):
                            nc.vector.tensor_tensor(
                                et[:, hp, a:b], et[:, hp, a:b],
                                mpat_sb[:, pi, a:b], MUL)
                    exp_tiles[kt] = et
                for hp in range(HPC):
                    av = avp.tile([65, SW], f32, tag="avp", name=f"av{n}{qs}{hp}")
                    for i, kt in enumerate(kts):
                        a, b = span_[qs][kt]
                        nc.tensor.matmul(
                            av[:, a:b],
                            vaug[n][:, kt * VW + hp * 65:kt * VW + (hp + 1) * 65],
                            exp_tiles[kt][:, hp, a:b],
                            start=(i == 0), stop=(i == len(kts) - 1))
                    den = nrm.tile([65, SW], bf16, tag="den", name=f"dn{n}{qs}{hp}")
                    nc.vector.tensor_copy(den[64:65, :], av[64:65, :])
                    bc = bcp.tile([DK, SW], f32, tag="bcp", name=f"bc{n}{qs}{hp}")
                    nc.tensor.matmul(bc[:], ones65[64:65, :], den[64:65, :],
                                     start=True, stop=True)
                    rec = nrm.tile([DK, SW], f32, tag="rec", name=f"rc{n}{qs}{hp}")
                    nc.vector.reciprocal(rec[:], bc[:])
                    nc.vector.tensor_tensor(
                        headT[n][hp][:, qs * SW:(qs + 1) * SW],
                        av[:DK, :], rec[:], MUL)

            def emit_a2a(n):
                for hp in range(HPC):
                    nc.sync.dma_start(
                        a2a_in[n][:, hp * DK:(hp + 1) * DK, :].transpose([1, 0, 2]),
                        headT[n][hp][:, :].rearrange("p (j c) -> p j c", j=NC))
                nc.gpsimd.collective_compute(
                    "AllToAll", mybir.AluOpType.bypass,
                    replica_groups=[list(range(NC))],
                    ins=[a2a_in[n].opt()], outs=[a2a_out[n].opt()])

            def emit_wo(n):
                rhs_t = wos.tile([128, NC, CB], bf16, tag="rhs", name=f"rhs{n}")
                nc.scalar.dma_start(rhs_t[:], a2a_out[n][:, :, :].transpose([1, 0, 2]))
                for mt in range(8):
                    ps = sp.tile([128, 1024], f32, tag="sp", name=f"wp{n}{mt}")
                    for jj in range(8):
                        nc.tensor.matmul(ps[:, :CB],
                                         wo_sb[:, jj, mt * KT:(mt + 1) * KT],
                                         rhs_t[:, jj, :],
                                         start=(jj == 0), stop=(jj == 7))
                    ob = osb.tile([128, CB], f32, tag="osb", name=f"ob{n}{mt}")
                    nc.vector.tensor_scalar_add(ob[:], ps[:, :CB], bo_sb[:, mt, :])
                    nc.scalar.dma_start(out_t[mt * KT:(mt + 1) * KT, n * CB:(n + 1) * CB],
                                        ob[:])

            # ---- pipeline ----
            emit_inputs(0)
            for b in range(NQS):
                emit_proj_block(0, b)
            emit_attn_qs(0, 0)
            emit_attn_qs(0, 1)
            emit_inputs(1)
            emit_proj_block(1, 0)
            emit_proj_block(1, 1)
            emit_attn_qs(0, 2)
            emit_proj_block(1, 2)
            emit_proj_block(1, 3)
            emit_attn_qs(0, 3)
            emit_a2a(0)
            for qs in range(NQS):
                emit_attn_qs(1, qs)
            emit_a2a(1)
            emit_wo(0)
            emit_wo(1)

    nc.compile()
    return nc


def kernel(x, y, mask, Wq, bq, Wk, bk, Wv, bv, Wo, bo, _trace=False):
    x = np.asarray(x, np.float32)
    y = np.asarray(y, np.float32)
    cls_, span_, pidx, pats = _classify_blocks(mask)

    key = (x.shape,
           tuple(tuple(c) for c in cls_),
           tuple(tuple(s) for s in span_),
           tuple(tuple(p) for p in pidx),
           pats.tobytes())
    if key not in _CACHE:
        _CACHE[key] = _build(cls_, span_, pidx, pats.shape[0])
    nc = _CACHE[key]

    fac = np.float32(1.0 / np.sqrt(DK))
    xtb = np.ascontiguousarray(
        np.concatenate([x[n].T for n in range(NB)], axis=1)).astype(BF16)
    ytb = np.ascontiguousarray(
        np.concatenate([y[n].T for n in range(NB)], axis=1)).astype(BF16)
    Wq32 = np.asarray(Wq, np.float32) * fac
    bq32 = np.asarray(bq, np.float32) * fac

    in_maps = []
    for c in range(NC):
        d0 = c * DPC
        wv_aug = np.zeros((DM, VW), np.float32)
        bv1 = np.zeros((1, VW), np.float32)
        for hp in range(HPC):
            h = HPC * c + hp
            wv_aug[:, hp * 65:hp * 65 + DK] = np.asarray(Wv, np.float32)[:, h * DK:(h + 1) * DK]
            bv1[0, hp * 65:hp * 65 + DK] = np.asarray(bv, np.float32)[h * DK:(h + 1) * DK]
            bv1[0, hp * 65 + DK] = 1.0
        in_maps.append({
            "xtb": xtb, "ytb": ytb,
            "wq": Wq32[:, d0:d0 + DPC].astype(BF16),
            "wk": np.asarray(Wk, np.float32)[:, d0:d0 + DPC].astype(BF16),
            "wv": wv_aug.astype(BF16),
            "wo": np.asarray(Wo, np.float32).astype(BF16),
            "bq": bq32[d0:d0 + DPC].reshape(DPC, 1),
            "bk": np.asarray(bk, np.float32)[d0:d0 + DPC].reshape(DPC, 1),
            "bv1": bv1.astype(BF16),
            "bo": np.asarray(bo, np.float32).reshape(DM, 1),
            "mpat": pats,
        })

    res = run_bass_kernel_spmd(nc, in_maps, core_ids=list(range(NC)), trace=_trace)
    out = np.empty((NB, L, DM), np.float32)
    for c in range(NC):
        for n in range(NB):
            out[n, c * CB:(c + 1) * CB, :] = res.results[c]["out_t"][:, n * CB:(n + 1) * CB].T
    if _trace:
        kernel.last_results = res
    return out


# revision 10
# speedup vs baseline: 1.3371x; 1.2241x over previous
"""Multi-head attention (N=2, L=2048, 16 heads x 64) on 8 TRN2 NeuronCores.

v2: head-parallel attention (2 heads/core) with a per-batch software pipeline:
load+project batch 0 -> attention batch 0 (overlapped with batch-1 loads and
projections) -> per-batch AllToAll (head-split -> sequence-split), each
overlapped with the next phase -> output projection per batch.

Key scheduling choices vs v1:
- Few, large DMAs with 3D access patterns (one per weight tensor, one per
  input quarter), split across the two HWDGE rings (sync + scalar).
- Softmax normalization without DMA: denominator row -> DVE reciprocal ->
  PE rank-1 broadcast matmul -> DVE multiply.
- Scores/exp/AV processed per 512-q sub-window; exp is one ScalarE call per
  k-tile covering both heads via a 3D access pattern.
- AllToAll split per batch so collective #0 hides under batch-1 attention
  and the batch-0 output projection hides under collective #1.
"""
import sys

sys.path.insert(0, "/opt/trn_rl_repo")

import numpy as np
import ml_dtypes

import concourse.bass as bass
import concourse.bacc as bacc
import concourse.mybir as mybir
import concourse.tile as tile
from concourse.bass_utils import run_bass_kernel_spmd

BF16 = ml_dtypes.bfloat16

DM = 1024      # dmodel
DK = 64        # head dim
H = 16         # heads
NB = 2         # batch
L = 2048       # seq len
R = NB * L
NC = 8         # cores
HPC = H // NC  # heads per core = 2
DPC = HPC * DK  # depth per core = 128

SW = 512       # q sub-window
KT = 128       # k tile
NQS = L // SW   # 4 q blocks per batch
NKT = L // KT   # 16 k tiles per batch
CB = L // NC    # 256: per-batch per-core output chunk
VW = 65 * HPC   # 130: augmented v width (both heads, +ones col each)

_CACHE = {}


def _classify_blocks(mask):
    """Per (qs, kt) block: 0=skip, 1=full, 2=partial (+ q-span, pattern)."""
    mask = np.asarray(mask, dtype=bool)
    cls = [[0] * NKT for _ in range(NQS)]
    span = [[None] * NKT for _ in range(NQS)]
    pat_ids = {}
    pats = []
    pat_idx = [[-1] * NKT for _ in range(NQS)]
    for qs in range(NQS):
        for kt in range(NKT):
            sub = mask[qs * SW:(qs + 1) * SW, kt * KT:(kt + 1) * KT]
            rows = np.nonzero(sub.any(axis=1))[0]
            if rows.size == 0:
                cls[qs][kt] = 0
            elif sub.all():
                cls[qs][kt] = 1
                span[qs][kt] = (0, SW)
            else:
                cls[qs][kt] = 2
                span[qs][kt] = (int(rows[0]), int(rows[-1]) + 1)
                pat = np.ascontiguousarray(sub.T).astype(BF16)  # [128 k, SW q]
                key = pat.tobytes()
                if key not in pat_ids:
                    pat_ids[key] = len(pats)
                    pats.append(pat)
                pat_idx[qs][kt] = pat_ids[key]
    # the first included kt of each sub-window must cover the full 512
    # columns (its start=True matmul clears PSUM has_written)
    for qs in range(NQS):
        for kt in range(NKT):
            if cls[qs][kt]:
                span[qs][kt] = (0, SW)
                break
    if not pats:
        pats.append(np.ones((KT, SW), dtype=BF16))
    return cls, span, pat_idx, np.stack(pats)


def _build(cls_, span_, pidx, n_pat):
    nc = bacc.Bacc("TRN2", target_bir_lowering=False, debug=False,
                   enable_asserts=False, num_devices=NC)
    f32, bf16 = mybir.dt.float32, mybir.dt.bfloat16
    EXP = mybir.ActivationFunctionType.Exp
    MUL = mybir.AluOpType.mult

    # weights arrive host-pre-shuffled into partition-major layouts so every
    # const DMA is one contiguous chunk per partition (few, large descriptors)
    xtb = nc.dram_tensor("xtb", [DM, R], bf16, kind="ExternalInput")
    ytb = nc.dram_tensor("ytb", [DM, R], bf16, kind="ExternalInput")
    wq = nc.dram_tensor("wq", [128, 8, DPC], bf16, kind="ExternalInput")
    wk = nc.dram_tensor("wk", [128, 8, DPC], bf16, kind="ExternalInput")
    wv = nc.dram_tensor("wv", [128, 8, VW], bf16, kind="ExternalInput")
    wo = nc.dram_tensor("wo", [128, 8, DM], bf16, kind="ExternalInput")
    bqd = nc.dram_tensor("bq", [DPC, 1], f32, kind="ExternalInput")
    bkd = nc.dram_tensor("bk", [DPC, 1], f32, kind="ExternalInput")
    bv1 = nc.dram_tensor("bv1", [1, VW], bf16, kind="ExternalInput")
    bod = nc.dram_tensor("bo", [128, 8, 1], f32, kind="ExternalInput")
    mpat = nc.dram_tensor("mpat", [KT, n_pat, SW], bf16, kind="ExternalInput")
    out_t = nc.dram_tensor("out_t", [DM, NB * CB], f32, kind="ExternalOutput")

    with tile.TileContext(nc) as tc:
        with (
            tc.tile_pool(name="cst", bufs=1) as cst,
            tc.tile_pool(name="xy", bufs=5) as xy,
            tc.tile_pool(name="big", bufs=1) as big,
            tc.tile_pool(name="expp", bufs=24) as expp,
            tc.tile_pool(name="nrm", bufs=3) as nrm,
            tc.tile_pool(name="wos", bufs=2) as wos,
            tc.tile_pool(name="osb", bufs=3) as osb,
            tc.tile_pool(name="sp", bufs=2, space="PSUM") as sp,
            tc.tile_pool(name="avp", bufs=2, space="PSUM") as avp,
            tc.tile_pool(name="bcp", bufs=2, space="PSUM") as bcp,
            tc.tile_pool(name="dram", bufs=1, space="DRAM") as dram,
        ):
            # ---- constants (scalar HWDGE ring; contiguous partition-major) ----
            bq_sb = cst.tile([DPC, 1], f32)
            bk_sb = cst.tile([DPC, 1], f32)
            bv1_sb = cst.tile([1, VW], bf16)
            bo_sb = cst.tile([128, 8, 1], f32)
            nc.scalar.dma_start(bk_sb[:], bkd[:])
            nc.scalar.dma_start(bq_sb[:], bqd[:])
            nc.scalar.dma_start(bv1_sb[:], bv1[:])
            nc.scalar.dma_start(bo_sb[:], bod[:, :, :])
            wq_sb = cst.tile([128, 8, DPC], bf16)
            wk_sb = cst.tile([128, 8, DPC], bf16)
            wv_sb = cst.tile([128, 8, VW], bf16)
            wo_sb = cst.tile([128, 8, DM], bf16)
            nc.scalar.dma_start(wk_sb[:], wk[:, :, :])
            nc.scalar.dma_start(wv_sb[:], wv[:, :, :])
            nc.scalar.dma_start(wq_sb[:], wq[:, :, :])
            mpat_sb = cst.tile([KT, n_pat, SW], bf16)
            nc.scalar.dma_start(mpat_sb[:], mpat[:, :, :])
            ones_row = cst.tile([1, 128], bf16)
            nc.vector.memset(ones_row[:], 1.0)
            ones65 = cst.tile([65, DK], bf16)
            nc.vector.memset(ones65[:], 1.0)

            # preload the exp table set during the DMA phase
            bar_sb = cst.tile([1, 8], f32)
            nc.vector.memset(bar_sb[:], 0.0)
            dum = cst.tile([1, 8], f32)
            nc.scalar.activation(dum[:], bar_sb[:], EXP)

            # ---- start-of-kernel barrier (absorbs launch skew) ----
            bar_in = dram.tile([1, 8], f32, tag="bar_in")
            bar_out = dram.tile([1, 8], f32, tag="bar_out")
            nc.sync.dma_start(bar_in[:], bar_sb[:])
            nc.gpsimd.collective_compute(
                "AllReduce", mybir.AluOpType.add,
                replica_groups=[list(range(NC))],
                ins=[bar_in.opt()], outs=[bar_out.opt()])

            qT = [big.tile([DPC, L], bf16, tag=f"qT{n}", name=f"qT{n}") for n in range(NB)]
            kT = [big.tile([DPC, L], bf16, tag=f"kT{n}", name=f"kT{n}") for n in range(NB)]
            vaug = [big.tile([128, NKT * VW], bf16, tag=f"va{n}", name=f"va{n}") for n in range(NB)]
            headT = [[big.tile([DK, L], bf16, tag=f"hT{n}{hp}", name=f"hT{n}{hp}")
                      for hp in range(HPC)] for n in range(NB)]

            a2a_in = [dram.tile([NC, DPC, CB], bf16, tag=f"a2ai{n}", name=f"a2ai{n}")
                      for n in range(NB)]
            a2a_out = [dram.tile([NC, DPC, CB], bf16, tag=f"a2ao{n}", name=f"a2ao{n}")
                       for n in range(NB)]

            ytile = [[None] * NQS for _ in range(NB)]
            xtile = [[None] * NQS for _ in range(NB)]

            def emit_inputs(n):
                for b in range(NQS):
                    yt = xy.tile([128, 8, SW], bf16, tag="xy", name=f"y{n}b{b}")
                    nc.sync.dma_start(
                        yt[:], ytb[:, n * L + b * SW:n * L + (b + 1) * SW]
                        .rearrange("(t p) c -> p t c", p=128))
                    ytile[n][b] = yt
                    xt = xy.tile([128, 8, SW], bf16, tag="xy", name=f"x{n}b{b}")
                    nc.sync.dma_start(
                        xt[:], xtb[:, n * L + b * SW:n * L + (b + 1) * SW]
                        .rearrange("(t p) c -> p t c", p=128))
                    xtile[n][b] = xt

            def emit_proj_block(n, b):
                yt = ytile[n][b]
                ps = sp.tile([128, 1024], f32, tag="sp", name=f"kp{n}{b}")
                for dt in range(8):
                    nc.tensor.matmul(ps[:DPC, :SW], wk_sb[:, dt, :], yt[:, dt, :],
                                     start=(dt == 0), stop=(dt == 7))
                nc.vector.tensor_scalar_add(kT[n][:, b * SW:(b + 1) * SW],
                                            ps[:DPC, :SW], bk_sb[:])
                for j in range(4):
                    kti = b * 4 + j
                    psv = sp.tile([128, 1024], f32, tag="sp", name=f"vp{n}{kti}")
                    for dt in range(8):
                        nc.tensor.matmul(psv[:, :VW], yt[:, dt, j * KT:(j + 1) * KT],
                                         wv_sb[:, dt, :], start=(dt == 0), stop=False)
                    nc.tensor.matmul(psv[:, :VW], ones_row[:], bv1_sb[:],
                                     start=False, stop=True)
                    nc.vector.tensor_copy(vaug[n][:, kti * VW:(kti + 1) * VW],
                                          psv[:, :VW])
                xt = xtile[n][b]
                ps2 = sp.tile([128, 1024], f32, tag="sp", name=f"qp{n}{b}")
                for dt in range(8):
                    nc.tensor.matmul(ps2[:DPC, :SW], wq_sb[:, dt, :], xt[:, dt, :],
                                     start=(dt == 0), stop=(dt == 7))
                nc.vector.tensor_scalar_add(qT[n][:, b * SW:(b + 1) * SW],
                                            ps2[:DPC, :SW], bq_sb[:])

            def emit_attn_qs(n, qs):
                kts = [kt for kt in range(NKT) if cls_[qs][kt]]
                exp_tiles = {}
                for kt in kts:
                    a, b = span_[qs][kt]
                    ps = sp.tile([128, 1024], f32, tag="sp", name=f"s{n}{qs}{kt}")
                    for hp in range(HPC):
                        hs = hp * DK
                        nc.tensor.matmul(
                            ps[:KT, hp * SW + a:hp * SW + b],
                            kT[n][hs:hs + DK, kt * KT:(kt + 1) * KT],
                            qT[n][hs:hs + DK, qs * SW + a:qs * SW + b],
                            start=True, stop=True)
                    et = expp.tile([128, 2, SW], bf16, tag="exp", name=f"e{n}{qs}{kt}")
                    nc.scalar.activation(
                        et[:, :, a:b],
                        ps.rearrange("p (h c) -> p h c", h=2)[:, :, a:b], EXP)
                    if cls_[qs][kt] == 2:
                        pi = pidx[qs][kt]
                        for hp in range(HPC):
                            nc.vector.tensor_tensor(
                                et[:, hp, a:b], et[:, hp, a:b],
                                mpat_sb[:, pi, a:b], MUL)
                    exp_tiles[kt] = et
                for hp in range(HPC):
                    av = avp.tile([65, SW], f32, tag="avp", name=f"av{n}{qs}{hp}")
                    for i, kt in enumerate(kts):
                        a, b = span_[qs][kt]
                        nc.tensor.matmul(
                            av[:, a:b],
                            vaug[n][:, kt * VW + hp * 65:kt * VW + (hp + 1) * 65],
                            exp_tiles[kt][:, hp, a:b],
                            start=(i == 0), stop=(i == len(kts) - 1))
                    den = nrm.tile([65, SW], bf16, tag="den", name=f"dn{n}{qs}{hp}")
                    nc.vector.tensor_copy(den[64:65, :], av[64:65, :])
                    bc = bcp.tile([DK, SW], f32, tag="bcp", name=f"bc{n}{qs}{hp}")
                    nc.tensor.matmul(bc[:], ones65[64:65, :], den[64:65, :],
                                     start=True, stop=True)
                    rec = nrm.tile([DK, SW], f32, tag="rec", name=f"rc{n}{qs}{hp}")
                    nc.vector.reciprocal_approx_fast(rec[:], bc[:])
                    nc.vector.tensor_tensor(
                        headT[n][hp][:, qs * SW:(qs + 1) * SW],
                        av[:DK, :], rec[:], MUL)

            def emit_a2a(n):
                for hp in range(HPC):
                    nc.sync.dma_start(
                        a2a_in[n][:, hp * DK:(hp + 1) * DK, :].transpose([1, 0, 2]),
                        headT[n][hp][:, :].rearrange("p (j c) -> p j c", j=NC))
                nc.gpsimd.collective_compute(
                    "AllToAll", mybir.AluOpType.bypass,
                    replica_groups=[list(range(NC))],
                    ins=[a2a_in[n].opt()], outs=[a2a_out[n].opt()])

            def emit_wo(n):
                rhs_t = wos.tile([128, NC, CB], bf16, tag="rhs", name=f"rhs{n}")
                nc.scalar.dma_start(rhs_t[:], a2a_out[n][:, :, :].transpose([1, 0, 2]))
                for mt in range(8):
                    ps = sp.tile([128, 1024], f32, tag="sp", name=f"wp{n}{mt}")
                    for jj in range(8):
                        nc.tensor.matmul(ps[:, :CB],
                                         wo_sb[:, jj, mt * KT:(mt + 1) * KT],
                                         rhs_t[:, jj, :],
                                         start=(jj == 0), stop=(jj == 7))
                    ob = osb.tile([128, CB], f32, tag="osb", name=f"ob{n}{mt}")
                    nc.vector.tensor_scalar_add(ob[:], ps[:, :CB], bo_sb[:, mt, :])
                    nc.scalar.dma_start(out_t[mt * KT:(mt + 1) * KT, n * CB:(n + 1) * CB],
                                        ob[:])

            # ---- pipeline ----
            emit_inputs(0)
            for b in range(NQS):
                emit_proj_block(0, b)
            emit_attn_qs(0, 0)
            emit_attn_qs(0, 1)
            emit_inputs(1)
            emit_proj_block(1, 0)
            emit_proj_block(1, 1)
            emit_attn_qs(0, 2)
            emit_proj_block(1, 2)
            emit_proj_block(1, 3)
            emit_attn_qs(0, 3)
            nc.scalar.dma_start(wo_sb[:], wo[:, :, :])
            emit_a2a(0)
            for qs in range(NQS):
                emit_attn_qs(1, qs)
            emit_a2a(1)
            emit_wo(0)
            emit_wo(1)

    nc.compile()
    return nc


def kernel(x, y, mask, Wq, bq, Wk, bk, Wv, bv, Wo, bo, _trace=False):
    x = np.asarray(x, np.float32)
    y = np.asarray(y, np.float32)
    cls_, span_, pidx, pats = _classify_blocks(mask)

    key = (x.shape,
           tuple(tuple(c) for c in cls_),
           tuple(tuple(s) for s in span_),
           tuple(tuple(p) for p in pidx),
           pats.tobytes())
    if key not in _CACHE:
        _CACHE[key] = _build(cls_, span_, pidx, pats.shape[0])
    nc = _CACHE[key]

    fac = np.float32(1.0 / np.sqrt(DK))
    xtb = np.ascontiguousarray(
        np.concatenate([x[n].T for n in range(NB)], axis=1)).astype(BF16)
    ytb = np.ascontiguousarray(
        np.concatenate([y[n].T for n in range(NB)], axis=1)).astype(BF16)
    Wq32 = np.asarray(Wq, np.float32) * fac
    bq32 = np.asarray(bq, np.float32) * fac

    def pmajor(w):
        # [DM, X] -> [128, 8, X] with [p, t, :] = w[t*128+p, :]
        w = np.asarray(w)
        return np.ascontiguousarray(w.reshape(8, 128, w.shape[1]).transpose(1, 0, 2))

    wo_pm = pmajor(np.asarray(Wo, np.float32)).astype(BF16)
    bo_pm = pmajor(np.asarray(bo, np.float32).reshape(DM, 1))
    mpat_t = np.ascontiguousarray(pats.transpose(1, 0, 2))

    in_maps = []
    for c in range(NC):
        d0 = c * DPC
        wv_aug = np.zeros((DM, VW), np.float32)
        bv1 = np.zeros((1, VW), np.float32)
        for hp in range(HPC):
            h = HPC * c + hp
            wv_aug[:, hp * 65:hp * 65 + DK] = np.asarray(Wv, np.float32)[:, h * DK:(h + 1) * DK]
            bv1[0, hp * 65:hp * 65 + DK] = np.asarray(bv, np.float32)[h * DK:(h + 1) * DK]
            bv1[0, hp * 65 + DK] = 1.0
        in_maps.append({
            "xtb": xtb, "ytb": ytb,
            "wq": pmajor(Wq32[:, d0:d0 + DPC]).astype(BF16),
            "wk": pmajor(np.asarray(Wk, np.float32)[:, d0:d0 + DPC]).astype(BF16),
            "wv": pmajor(wv_aug).astype(BF16),
            "wo": wo_pm,
            "bq": bq32[d0:d0 + DPC].reshape(DPC, 1),
            "bk": np.asarray(bk, np.float32)[d0:d0 + DPC].reshape(DPC, 1),
            "bv1": bv1.astype(BF16),
            "bo": bo_pm,
            "mpat": mpat_t,
        })

    res = run_bass_kernel_spmd(nc, in_maps, core_ids=list(range(NC)), trace=_trace)
    out = np.empty((NB, L, DM), np.float32)
    for c in range(NC):
        for n in range(NB):
            out[n, c * CB:(c + 1) * CB, :] = res.results[c]["out_t"][:, n * CB:(n + 1) * CB].T
    if _trace:
        kernel.last_results = res
    return out


# revision 16
# speedup vs baseline: 1.3485x; 1.0085x over previous
"""Multi-head attention (N=2, L=2048, 16 heads x 64) on 8 TRN2 NeuronCores.

v2: head-parallel attention (2 heads/core) with a per-batch software pipeline:
load+project batch 0 -> attention batch 0 (overlapped with batch-1 loads and
projections) -> per-batch AllToAll (head-split -> sequence-split), each
overlapped with the next phase -> output projection per batch.

Key scheduling choices vs v1:
- Few, large DMAs with 3D access patterns (one per weight tensor, one per
  input quarter), split across the two HWDGE rings (sync + scalar).
- Softmax normalization without DMA: denominator row -> DVE reciprocal ->
  PE rank-1 broadcast matmul -> DVE multiply.
- Scores/exp/AV processed per 512-q sub-window; exp is one ScalarE call per
  k-tile covering both heads via a 3D access pattern.
- AllToAll split per batch so collective #0 hides under batch-1 attention
  and the batch-0 output projection hides under collective #1.
"""
import sys

sys.path.insert(0, "/opt/trn_rl_repo")

import numpy as np
import ml_dtypes

import concourse.bass as bass
import concourse.bacc as bacc
import concourse.mybir as mybir
import concourse.tile as tile
from concourse.bass_utils import run_bass_kernel_spmd

BF16 = ml_dtypes.bfloat16

DM = 1024      # dmodel
DK = 64        # head dim
H = 16         # heads
NB = 2         # batch
L = 2048       # seq len
R = NB * L
NC = 8         # cores
HPC = H // NC  # heads per core = 2
DPC = HPC * DK  # depth per core = 128

SW = 512       # q sub-window
KT = 128       # k tile
NQS = L // SW   # 4 q blocks per batch
NKT = L // KT   # 16 k tiles per batch
CB = L // NC    # 256: per-batch per-core output chunk
VW = 65 * HPC   # 130: augmented v width (both heads, +ones col each)

_CACHE = {}


def _classify_blocks(mask):
    """Per (qs, kt) block: 0=skip, 1=full, 2=partial (+ q-span, pattern)."""
    mask = np.asarray(mask, dtype=bool)
    cls = [[0] * NKT for _ in range(NQS)]
    span = [[None] * NKT for _ in range(NQS)]
    pat_ids = {}
    pats = []
    pat_idx = [[-1] * NKT for _ in range(NQS)]
    for qs in range(NQS):
        for kt in range(NKT):
            sub = mask[qs * SW:(qs + 1) * SW, kt * KT:(kt + 1) * KT]
            rows = np.nonzero(sub.any(axis=1))[0]
            if rows.size == 0:
                cls[qs][kt] = 0
            elif sub.all():
                cls[qs][kt] = 1
                span[qs][kt] = (0, SW)
            else:
                cls[qs][kt] = 2
                span[qs][kt] = (int(rows[0]), int(rows[-1]) + 1)
                pat = np.ascontiguousarray(sub.T).astype(BF16)  # [128 k, SW q]
                key = pat.tobytes()
                if key not in pat_ids:
                    pat_ids[key] = len(pats)
                    pats.append(pat)
                pat_idx[qs][kt] = pat_ids[key]
    # the first included kt of each sub-window must cover the full 512
    # columns (its start=True matmul clears PSUM has_written)
    for qs in range(NQS):
        for kt in range(NKT):
            if cls[qs][kt]:
                span[qs][kt] = (0, SW)
                break
    if not pats:
        pats.append(np.ones((KT, SW), dtype=BF16))
    return cls, span, pat_idx, np.stack(pats)


def _build(cls_, span_, pidx, n_pat):
    nc = bacc.Bacc("TRN2", target_bir_lowering=False, debug=False,
                   enable_asserts=False, num_devices=NC)
    f32, bf16 = mybir.dt.float32, mybir.dt.bfloat16
    EXP = mybir.ActivationFunctionType.Exp
    MUL = mybir.AluOpType.mult

    # weights arrive host-pre-shuffled into partition-major layouts so every
    # const DMA is one contiguous chunk per partition (few, large descriptors)
    xtb = nc.dram_tensor("xtb", [DM, R], bf16, kind="ExternalInput")
    ytb = nc.dram_tensor("ytb", [DM, R], bf16, kind="ExternalInput")
    wq = nc.dram_tensor("wq", [128, 8, DPC], bf16, kind="ExternalInput")
    wk = nc.dram_tensor("wk", [128, 8, DPC], bf16, kind="ExternalInput")
    wv = nc.dram_tensor("wv", [128, 8, VW], bf16, kind="ExternalInput")
    wo = nc.dram_tensor("wo", [128, 8, DM], bf16, kind="ExternalInput")
    bqd = nc.dram_tensor("bq", [DPC, 1], f32, kind="ExternalInput")
    bkd = nc.dram_tensor("bk", [DPC, 1], f32, kind="ExternalInput")
    bv1 = nc.dram_tensor("bv1", [1, VW], bf16, kind="ExternalInput")
    bod = nc.dram_tensor("bo", [128, 8, 1], f32, kind="ExternalInput")
    mpat = nc.dram_tensor("mpat", [KT, n_pat, SW], bf16, kind="ExternalInput")
    out_t = nc.dram_tensor("out_t", [DM, NB * CB], f32, kind="ExternalOutput")

    with tile.TileContext(nc) as tc:
        with (
            tc.tile_pool(name="cst", bufs=1) as cst,
            tc.tile_pool(name="xy", bufs=5) as xy,
            tc.tile_pool(name="big", bufs=1) as big,
            tc.tile_pool(name="expp", bufs=24) as expp,
            tc.tile_pool(name="nrm", bufs=3) as nrm,
            tc.tile_pool(name="wos", bufs=2) as wos,
            tc.tile_pool(name="osb", bufs=3) as osb,
            tc.tile_pool(name="sp", bufs=2, space="PSUM") as sp,
            tc.tile_pool(name="avp", bufs=2, space="PSUM") as avp,
            tc.tile_pool(name="bcp", bufs=2, space="PSUM") as bcp,
            tc.tile_pool(name="dram", bufs=1, space="DRAM") as dram,
        ):
            # ---- constants (scalar HWDGE ring; contiguous partition-major) ----
            bq_sb = cst.tile([DPC, 1], f32)
            bk_sb = cst.tile([DPC, 1], f32)
            bv1_sb = cst.tile([1, VW], bf16)
            bo_sb = cst.tile([128, 8, 1], f32)
            nc.scalar.dma_start(bk_sb[:], bkd[:])
            nc.scalar.dma_start(bq_sb[:], bqd[:])
            nc.scalar.dma_start(bv1_sb[:], bv1[:])
            nc.scalar.dma_start(bo_sb[:], bod[:, :, :])
            wq_sb = cst.tile([128, 8, DPC], bf16)
            wk_sb = cst.tile([128, 8, DPC], bf16)
            wv_sb = cst.tile([128, 8, VW], bf16)
            wo_sb = cst.tile([128, 8, DM], bf16)
            nc.scalar.dma_start(wk_sb[:], wk[:, :, :])
            nc.scalar.dma_start(wv_sb[:], wv[:, :, :])
            nc.scalar.dma_start(wq_sb[:], wq[:, :, :])
            mpat_sb = cst.tile([KT, n_pat, SW], bf16)
            nc.scalar.dma_start(mpat_sb[:], mpat[:, :, :])
            ones_row = cst.tile([1, 128], bf16)
            nc.vector.memset(ones_row[:], 1.0)
            ones65 = cst.tile([65, DK], bf16)
            nc.vector.memset(ones65[:], 1.0)

            # preload the exp table set during the DMA phase
            bar_sb = cst.tile([1, 8], f32)
            nc.vector.memset(bar_sb[:], 0.0)
            dum = cst.tile([1, 8], f32)
            nc.scalar.activation(dum[:], bar_sb[:], EXP)

            # ---- start-of-kernel barrier (absorbs launch skew) ----
            bar_in = dram.tile([1, 8], f32, tag="bar_in")
            bar_out = dram.tile([1, 8], f32, tag="bar_out")
            nc.sync.dma_start(bar_in[:], bar_sb[:])
            nc.gpsimd.collective_compute(
                "AllReduce", mybir.AluOpType.add,
                replica_groups=[list(range(NC))],
                ins=[bar_in.opt()], outs=[bar_out.opt()])

            qT = [big.tile([DPC, L], bf16, tag=f"qT{n}", name=f"qT{n}") for n in range(NB)]
            kT = [big.tile([DPC, L], bf16, tag=f"kT{n}", name=f"kT{n}") for n in range(NB)]
            vaug = [big.tile([128, NKT * VW], bf16, tag=f"va{n}", name=f"va{n}") for n in range(NB)]
            headT = [[big.tile([DK, L], bf16, tag=f"hT{n}{hp}", name=f"hT{n}{hp}")
                      for hp in range(HPC)] for n in range(NB)]

            a2a_in = [dram.tile([NC, DPC, CB], bf16, tag=f"a2ai{n}", name=f"a2ai{n}")
                      for n in range(NB)]
            a2a_out = [dram.tile([NC, DPC, CB], bf16, tag=f"a2ao{n}", name=f"a2ao{n}")
                       for n in range(NB)]

            ytile = [[None] * NQS for _ in range(NB)]
            xtile = [[None] * NQS for _ in range(NB)]

            def emit_inputs(n):
                for b in range(NQS):
                    yt = xy.tile([128, 8, SW], bf16, tag="xy", name=f"y{n}b{b}")
                    nc.sync.dma_start(
                        yt[:], ytb[:, n * L + b * SW:n * L + (b + 1) * SW]
                        .rearrange("(t p) c -> p t c", p=128))
                    ytile[n][b] = yt
                    xt = xy.tile([128, 8, SW], bf16, tag="xy", name=f"x{n}b{b}")
                    nc.sync.dma_start(
                        xt[:], xtb[:, n * L + b * SW:n * L + (b + 1) * SW]
                        .rearrange("(t p) c -> p t c", p=128))
                    xtile[n][b] = xt

            def _v_chain(n, b, j, psv, dt, last):
                # one step of a V-projection chain (stationary = y k-tile)
                yt = ytile[n][b]
                if dt < 8:
                    nc.tensor.matmul(psv[:, :VW], yt[:, dt, j * KT:(j + 1) * KT],
                                     wv_sb[:, dt, :], start=(dt == 0), stop=False)
                else:
                    nc.tensor.matmul(psv[:, :VW], ones_row[:], bv1_sb[:],
                                     start=False, stop=True)

            def emit_proj_block(n, b):
                # chains interleaved in PAIRS so consecutive matmuls hit
                # alternating PSUM banks (avoids same-bank drain stalls)
                yt = ytile[n][b]
                xt = xtile[n][b]
                # pair 1: K chain & V chain j=0
                ps_k = sp.tile([128, 1024], f32, tag="sp", name=f"kp{n}{b}")
                psv0 = sp.tile([128, 1024], f32, tag="sp", name=f"vp{n}{b}0")
                for dt in range(9):
                    if dt < 8:
                        nc.tensor.matmul(ps_k[:DPC, :SW], wk_sb[:, dt, :], yt[:, dt, :],
                                         start=(dt == 0), stop=(dt == 7))
                    _v_chain(n, b, 0, psv0, dt, False)
                nc.vector.tensor_scalar_add(kT[n][:, b * SW:(b + 1) * SW],
                                            ps_k[:DPC, :SW], bk_sb[:])
                nc.vector.tensor_copy(vaug[n][:, (b * 4) * VW:(b * 4 + 1) * VW],
                                      psv0[:, :VW])
                # pair 2: V chains j=1 & j=2
                psv1 = sp.tile([128, 1024], f32, tag="sp", name=f"vp{n}{b}1")
                psv2 = sp.tile([128, 1024], f32, tag="sp", name=f"vp{n}{b}2")
                for dt in range(9):
                    _v_chain(n, b, 1, psv1, dt, False)
                    _v_chain(n, b, 2, psv2, dt, False)
                nc.vector.tensor_copy(vaug[n][:, (b * 4 + 1) * VW:(b * 4 + 2) * VW],
                                      psv1[:, :VW])
                nc.vector.tensor_copy(vaug[n][:, (b * 4 + 2) * VW:(b * 4 + 3) * VW],
                                      psv2[:, :VW])
                # pair 3: V chain j=3 & Q chain
                psv3 = sp.tile([128, 1024], f32, tag="sp", name=f"vp{n}{b}3")
                ps_q = sp.tile([128, 1024], f32, tag="sp", name=f"qp{n}{b}")
                for dt in range(9):
                    _v_chain(n, b, 3, psv3, dt, False)
                    if dt < 8:
                        nc.tensor.matmul(ps_q[:DPC, :SW], wq_sb[:, dt, :], xt[:, dt, :],
                                         start=(dt == 0), stop=(dt == 7))
                nc.vector.tensor_copy(vaug[n][:, (b * 4 + 3) * VW:(b * 4 + 4) * VW],
                                      psv3[:, :VW])
                nc.vector.tensor_scalar_add(qT[n][:, b * SW:(b + 1) * SW],
                                            ps_q[:DPC, :SW], bq_sb[:])

            def emit_attn_qs(n, qs):
                kts = [kt for kt in range(NKT) if cls_[qs][kt]]
                exp_tiles = {}
                for kt in kts:
                    a, b = span_[qs][kt]
                    ps = sp.tile([128, 1024], f32, tag="sp", name=f"s{n}{qs}{kt}")
                    for hp in range(HPC):
                        hs = hp * DK
                        nc.tensor.matmul(
                            ps[:KT, hp * SW + a:hp * SW + b],
                            kT[n][hs:hs + DK, kt * KT:(kt + 1) * KT],
                            qT[n][hs:hs + DK, qs * SW + a:qs * SW + b],
                            start=True, stop=True)
                    et = expp.tile([128, 2, SW], bf16, tag="exp", name=f"e{n}{qs}{kt}")
                    nc.scalar.activation(
                        et[:, :, a:b],
                        ps.rearrange("p (h c) -> p h c", h=2)[:, :, a:b], EXP)
                    if cls_[qs][kt] == 2:
                        pi = pidx[qs][kt]
                        for hp in range(HPC):
                            nc.vector.tensor_tensor(
                                et[:, hp, a:b], et[:, hp, a:b],
                                mpat_sb[:, pi, a:b], MUL)
                    exp_tiles[kt] = et
                # AV: the two head chains interleaved so consecutive matmuls
                # alternate PSUM banks (avoids same-bank drain stalls)
                avs = [avp.tile([65, SW], f32, tag="avp", name=f"av{n}{qs}{hp}")
                       for hp in range(HPC)]
                for i, kt in enumerate(kts):
                    a, b = span_[qs][kt]
                    for hp in range(HPC):
                        nc.tensor.matmul(
                            avs[hp][:, a:b],
                            vaug[n][:, kt * VW + hp * 65:kt * VW + (hp + 1) * 65],
                            exp_tiles[kt][:, hp, a:b],
                            start=(i == 0), stop=(i == len(kts) - 1))
                for hp in range(HPC):
                    av = avs[hp]
                    den = nrm.tile([65, SW], bf16, tag="den", name=f"dn{n}{qs}{hp}")
                    nc.vector.tensor_copy(den[64:65, :], av[64:65, :])
                    bc = bcp.tile([DK, SW], f32, tag="bcp", name=f"bc{n}{qs}{hp}")
                    nc.tensor.matmul(bc[:], ones65[64:65, :], den[64:65, :],
                                     start=True, stop=True)
                    rec = nrm.tile([DK, SW], f32, tag="rec", name=f"rc{n}{qs}{hp}")
                    nc.vector.reciprocal_approx_fast(rec[:], bc[:])
                    nc.vector.tensor_tensor(
                        headT[n][hp][:, qs * SW:(qs + 1) * SW],
                        av[:DK, :], rec[:], MUL)

            def emit_a2a(n):
                for hp in range(HPC):
                    nc.sync.dma_start(
                        a2a_in[n][:, hp * DK:(hp + 1) * DK, :].transpose([1, 0, 2]),
                        headT[n][hp][:, :].rearrange("p (j c) -> p j c", j=NC))
                nc.gpsimd.collective_compute(
                    "AllToAll", mybir.AluOpType.bypass,
                    replica_groups=[list(range(NC))],
                    ins=[a2a_in[n].opt()], outs=[a2a_out[n].opt()])

            def emit_wo(n):
                rhs_t = wos.tile([128, NC, CB], bf16, tag="rhs", name=f"rhs{n}")
                nc.scalar.dma_start(rhs_t[:], a2a_out[n][:, :, :].transpose([1, 0, 2]))
                for mt0 in range(0, 8, 2):
                    pss = [sp.tile([128, 1024], f32, tag="sp", name=f"wp{n}{mt0}{k}")
                           for k in range(2)]
                    for jj in range(8):
                        for k in range(2):
                            mt = mt0 + k
                            nc.tensor.matmul(pss[k][:, :CB],
                                             wo_sb[:, jj, mt * KT:(mt + 1) * KT],
                                             rhs_t[:, jj, :],
                                             start=(jj == 0), stop=(jj == 7))
                    for k in range(2):
                        mt = mt0 + k
                        ob = osb.tile([128, CB], f32, tag="osb", name=f"ob{n}{mt}")
                        nc.vector.tensor_scalar_add(ob[:], pss[k][:, :CB], bo_sb[:, mt, :])
                        nc.scalar.dma_start(out_t[mt * KT:(mt + 1) * KT, n * CB:(n + 1) * CB],
                                            ob[:])

            # ---- pipeline ----
            emit_inputs(0)
            for b in range(NQS):
                emit_proj_block(0, b)
            emit_attn_qs(0, 0)
            emit_attn_qs(0, 1)
            emit_inputs(1)
            emit_proj_block(1, 0)
            emit_proj_block(1, 1)
            emit_attn_qs(0, 2)
            emit_proj_block(1, 2)
            emit_proj_block(1, 3)
            emit_attn_qs(0, 3)
            nc.scalar.dma_start(wo_sb[:], wo[:, :, :])
            emit_a2a(0)
            for qs in range(NQS):
                emit_attn_qs(1, qs)
            emit_a2a(1)
            emit_wo(0)
            emit_wo(1)

    nc.compile()
    return nc


def kernel(x, y, mask, Wq, bq, Wk, bk, Wv, bv, Wo, bo, _trace=False):
    x = np.asarray(x, np.float32)
    y = np.asarray(y, np.float32)
    cls_, span_, pidx, pats = _classify_blocks(mask)

    key = (x.shape,
           tuple(tuple(c) for c in cls_),
           tuple(tuple(s) for s in span_),
           tuple(tuple(p) for p in pidx),
           pats.tobytes())
    if key not in _CACHE:
        _CACHE[key] = _build(cls_, span_, pidx, pats.shape[0])
    nc = _CACHE[key]

    fac = np.float32(1.0 / np.sqrt(DK))
    xtb = np.ascontiguousarray(
        np.concatenate([x[n].T for n in range(NB)], axis=1)).astype(BF16)
    ytb = np.ascontiguousarray(
        np.concatenate([y[n].T for n in range(NB)], axis=1)).astype(BF16)
    Wq32 = np.asarray(Wq, np.float32) * fac
    bq32 = np.asarray(bq, np.float32) * fac

    def pmajor(w):
        # [DM, X] -> [128, 8, X] with [p, t, :] = w[t*128+p, :]
        w = np.asarray(w)
        return np.ascontiguousarray(w.reshape(8, 128, w.shape[1]).transpose(1, 0, 2))

    wo_pm = pmajor(np.asarray(Wo, np.float32)).astype(BF16)
    bo_pm = pmajor(np.asarray(bo, np.float32).reshape(DM, 1))
    mpat_t = np.ascontiguousarray(pats.transpose(1, 0, 2))

    in_maps = []
    for c in range(NC):
        d0 = c * DPC
        wv_aug = np.zeros((DM, VW), np.float32)
        bv1 = np.zeros((1, VW), np.float32)
        for hp in range(HPC):
            h = HPC * c + hp
            wv_aug[:, hp * 65:hp * 65 + DK] = np.asarray(Wv, np.float32)[:, h * DK:(h + 1) * DK]
            bv1[0, hp * 65:hp * 65 + DK] = np.asarray(bv, np.float32)[h * DK:(h + 1) * DK]
            bv1[0, hp * 65 + DK] = 1.0
        in_maps.append({
            "xtb": xtb, "ytb": ytb,
            "wq": pmajor(Wq32[:, d0:d0 + DPC]).astype(BF16),
            "wk": pmajor(np.asarray(Wk, np.float32)[:, d0:d0 + DPC]).astype(BF16),
            "wv": pmajor(wv_aug).astype(BF16),
            "wo": wo_pm,
            "bq": bq32[d0:d0 + DPC].reshape(DPC, 1),
            "bk": np.asarray(bk, np.float32)[d0:d0 + DPC].reshape(DPC, 1),
            "bv1": bv1.astype(BF16),
            "bo": bo_pm,
            "mpat": mpat_t,
        })

    res = run_bass_kernel_spmd(nc, in_maps, core_ids=list(range(NC)), trace=_trace)
    out = np.empty((NB, L, DM), np.float32)
    for c in range(NC):
        for n in range(NB):
            out[n, c * CB:(c + 1) * CB, :] = res.results[c]["out_t"][:, n * CB:(n + 1) * CB].T
    if _trace:
        kernel.last_results = res
    return out


# revision 20
# speedup vs baseline: 1.4011x; 1.0390x over previous
"""Multi-head attention (N=2, L=2048, 16 heads x 64) on 8 TRN2 NeuronCores.

v2: head-parallel attention (2 heads/core) with a per-batch software pipeline:
load+project batch 0 -> attention batch 0 (overlapped with batch-1 loads and
projections) -> per-batch AllToAll (head-split -> sequence-split), each
overlapped with the next phase -> output projection per batch.

Key scheduling choices vs v1:
- Few, large DMAs with 3D access patterns (one per weight tensor, one per
  input quarter), split across the two HWDGE rings (sync + scalar).
- Softmax normalization without DMA: denominator row -> DVE reciprocal ->
  PE rank-1 broadcast matmul -> DVE multiply.
- Scores/exp/AV processed per 512-q sub-window; exp is one ScalarE call per
  k-tile covering both heads via a 3D access pattern.
- AllToAll split per batch so collective #0 hides under batch-1 attention
  and the batch-0 output projection hides under collective #1.
"""
import sys

sys.path.insert(0, "/opt/trn_rl_repo")

import numpy as np
import ml_dtypes

import concourse.bass as bass
import concourse.bacc as bacc
import concourse.mybir as mybir
import concourse.tile as tile
from concourse.bass_utils import run_bass_kernel_spmd

BF16 = ml_dtypes.bfloat16

DM = 1024      # dmodel
DK = 64        # head dim
H = 16         # heads
NB = 2         # batch
L = 2048       # seq len
R = NB * L
NC = 8         # cores
HPC = H // NC  # heads per core = 2
DPC = HPC * DK  # depth per core = 128

SW = 512       # q sub-window
KT = 128       # k tile
NQS = L // SW   # 4 q blocks per batch
NKT = L // KT   # 16 k tiles per batch
CB = L // NC    # 256: per-batch per-core output chunk
VW = 65 * HPC   # 130: augmented v width (both heads, +ones col each)

_CACHE = {}


def _classify_blocks(mask):
    """Per (qs, kt) block: 0=skip, 1=full, 2=partial (+ q-span, pattern)."""
    mask = np.asarray(mask, dtype=bool)
    cls = [[0] * NKT for _ in range(NQS)]
    span = [[None] * NKT for _ in range(NQS)]
    pat_ids = {}
    pats = []
    pat_idx = [[-1] * NKT for _ in range(NQS)]
    for qs in range(NQS):
        for kt in range(NKT):
            sub = mask[qs * SW:(qs + 1) * SW, kt * KT:(kt + 1) * KT]
            rows = np.nonzero(sub.any(axis=1))[0]
            if rows.size == 0:
                cls[qs][kt] = 0
            elif sub.all():
                cls[qs][kt] = 1
                span[qs][kt] = (0, SW)
            else:
                cls[qs][kt] = 2
                span[qs][kt] = (int(rows[0]), int(rows[-1]) + 1)
                pat = np.ascontiguousarray(sub.T).astype(BF16)  # [128 k, SW q]
                key = pat.tobytes()
                if key not in pat_ids:
                    pat_ids[key] = len(pats)
                    pats.append(pat)
                pat_idx[qs][kt] = pat_ids[key]
    # the first included kt of each sub-window must cover the full 512
    # columns (its start=True matmul clears PSUM has_written)
    for qs in range(NQS):
        for kt in range(NKT):
            if cls[qs][kt]:
                span[qs][kt] = (0, SW)
                break
    if not pats:
        pats.append(np.ones((KT, SW), dtype=BF16))
    return cls, span, pat_idx, np.stack(pats)


def _build(cls_, span_, pidx, n_pat):
    nc = bacc.Bacc("TRN2", target_bir_lowering=False, debug=False,
                   enable_asserts=False, num_devices=NC)
    f32, bf16 = mybir.dt.float32, mybir.dt.bfloat16
    EXP = mybir.ActivationFunctionType.Exp
    MUL = mybir.AluOpType.mult

    # weights arrive host-pre-shuffled into partition-major layouts so every
    # const DMA is one contiguous chunk per partition (few, large descriptors)
    xtb = nc.dram_tensor("xtb", [DM, R], bf16, kind="ExternalInput")
    ytb = nc.dram_tensor("ytb", [DM, R], bf16, kind="ExternalInput")
    wq = nc.dram_tensor("wq", [128, 8, DPC], bf16, kind="ExternalInput")
    wk = nc.dram_tensor("wk", [128, 8, DPC], bf16, kind="ExternalInput")
    wv = nc.dram_tensor("wv", [128, 8, VW], bf16, kind="ExternalInput")
    wo = nc.dram_tensor("wo", [128, 8, DM], bf16, kind="ExternalInput")
    bqd = nc.dram_tensor("bq", [DPC, 1], f32, kind="ExternalInput")
    bkd = nc.dram_tensor("bk", [DPC, 1], f32, kind="ExternalInput")
    bv1 = nc.dram_tensor("bv1", [1, VW], bf16, kind="ExternalInput")
    bod = nc.dram_tensor("bo", [128, 8, 1], f32, kind="ExternalInput")
    mpat = nc.dram_tensor("mpat", [KT, n_pat, SW], bf16, kind="ExternalInput")
    out_t = nc.dram_tensor("out_t", [DM, NB * CB], f32, kind="ExternalOutput")

    with tile.TileContext(nc) as tc:
        with (
            tc.tile_pool(name="cst", bufs=1) as cst,
            tc.tile_pool(name="xy", bufs=6) as xy,
            tc.tile_pool(name="big", bufs=1) as big,
            tc.tile_pool(name="expp", bufs=24) as expp,
            tc.tile_pool(name="nrm", bufs=3) as nrm,
            tc.tile_pool(name="wos", bufs=2) as wos,
            tc.tile_pool(name="osb", bufs=3) as osb,
            tc.tile_pool(name="sp", bufs=3, space="PSUM") as sp,
            tc.tile_pool(name="avp", bufs=2, space="PSUM") as avp,
            tc.tile_pool(name="dram", bufs=1, space="DRAM") as dram,
        ):
            # ---- constants (scalar HWDGE ring; contiguous partition-major) ----
            bq_sb = cst.tile([DPC, 1], f32)
            bk_sb = cst.tile([DPC, 1], f32)
            bv1_sb = cst.tile([1, VW], bf16)
            bo_sb = cst.tile([128, 8, 1], f32)
            nc.scalar.dma_start(bk_sb[:], bkd[:])
            nc.scalar.dma_start(bq_sb[:], bqd[:])
            nc.scalar.dma_start(bv1_sb[:], bv1[:])
            nc.scalar.dma_start(bo_sb[:], bod[:, :, :])
            wq_sb = cst.tile([128, 8, DPC], bf16)
            wk_sb = cst.tile([128, 8, DPC], bf16)
            wv_sb = cst.tile([128, 8, VW], bf16)
            wo_sb = cst.tile([128, 8, DM], bf16)
            nc.scalar.dma_start(wk_sb[:], wk[:, :, :])
            nc.scalar.dma_start(wv_sb[:], wv[:, :, :])
            nc.scalar.dma_start(wq_sb[:], wq[:, :, :])
            mpat_sb = cst.tile([KT, n_pat, SW], bf16)
            nc.scalar.dma_start(mpat_sb[:], mpat[:, :, :])
            ones_row = cst.tile([1, 128], bf16)
            nc.vector.memset(ones_row[:], 1.0)
            ones65 = cst.tile([65, DK], bf16)
            nc.vector.memset(ones65[:], 1.0)

            # preload the exp table set during the DMA phase
            bar_sb = cst.tile([1, 8], f32)
            nc.vector.memset(bar_sb[:], 0.0)
            dum = cst.tile([1, 8], f32)
            nc.scalar.activation(dum[:], bar_sb[:], EXP)

            # ---- start-of-kernel barrier (absorbs launch skew) ----
            bar_in = dram.tile([1, 8], f32, tag="bar_in")
            bar_out = dram.tile([1, 8], f32, tag="bar_out")
            nc.sync.dma_start(bar_in[:], bar_sb[:])
            nc.gpsimd.collective_compute(
                "AllReduce", mybir.AluOpType.add,
                replica_groups=[list(range(NC))],
                ins=[bar_in.opt()], outs=[bar_out.opt()])

            qT = [big.tile([DPC, L], bf16, tag=f"qT{n}", name=f"qT{n}") for n in range(NB)]
            kT = [big.tile([DPC, L], bf16, tag=f"kT{n}", name=f"kT{n}") for n in range(NB)]
            vaug = [big.tile([128, NKT * VW], bf16, tag=f"va{n}", name=f"va{n}") for n in range(NB)]
            headT = [[big.tile([DK, L], bf16, tag=f"hT{n}{hp}", name=f"hT{n}{hp}")
                      for hp in range(HPC)] for n in range(NB)]

            a2a_in = [dram.tile([NC, DPC, CB], bf16, tag=f"a2ai{n}", name=f"a2ai{n}")
                      for n in range(NB)]
            a2a_out = [dram.tile([NC, DPC, CB], bf16, tag=f"a2ao{n}", name=f"a2ao{n}")
                       for n in range(NB)]

            ytile = [[None] * NQS for _ in range(NB)]
            xtile = [[None] * NQS for _ in range(NB)]

            def emit_inputs(n):
                for b in range(NQS):
                    yt = xy.tile([128, 8, SW], bf16, tag="xy", name=f"y{n}b{b}")
                    nc.sync.dma_start(
                        yt[:], ytb[:, n * L + b * SW:n * L + (b + 1) * SW]
                        .rearrange("(t p) c -> p t c", p=128))
                    ytile[n][b] = yt
                    xt = xy.tile([128, 8, SW], bf16, tag="xy", name=f"x{n}b{b}")
                    nc.sync.dma_start(
                        xt[:], xtb[:, n * L + b * SW:n * L + (b + 1) * SW]
                        .rearrange("(t p) c -> p t c", p=128))
                    xtile[n][b] = xt

            def _v_chain(n, b, j, psv, dt, last):
                # one step of a V-projection chain (stationary = y k-tile)
                yt = ytile[n][b]
                if dt < 8:
                    nc.tensor.matmul(psv[:, :VW], yt[:, dt, j * KT:(j + 1) * KT],
                                     wv_sb[:, dt, :], start=(dt == 0), stop=False)
                else:
                    nc.tensor.matmul(psv[:, :VW], ones_row[:], bv1_sb[:],
                                     start=False, stop=True)

            def emit_proj_block(n, b):
                # chains interleaved in PAIRS so consecutive matmuls hit
                # alternating PSUM banks (avoids same-bank drain stalls)
                yt = ytile[n][b]
                xt = xtile[n][b]
                # pair 1: K chain & V chain j=0
                ps_k = sp.tile([128, 1024], f32, tag="sp", name=f"kp{n}{b}")
                psv0 = sp.tile([128, 1024], f32, tag="sp", name=f"vp{n}{b}0")
                for dt in range(9):
                    if dt < 8:
                        nc.tensor.matmul(ps_k[:DPC, :SW], wk_sb[:, dt, :], yt[:, dt, :],
                                         start=(dt == 0), stop=(dt == 7))
                    _v_chain(n, b, 0, psv0, dt, False)
                nc.vector.tensor_scalar_add(kT[n][:, b * SW:(b + 1) * SW],
                                            ps_k[:DPC, :SW], bk_sb[:])
                nc.vector.tensor_copy(vaug[n][:, (b * 4) * VW:(b * 4 + 1) * VW],
                                      psv0[:, :VW])
                # pair 2: V chains j=1 & j=2
                psv1 = sp.tile([128, 1024], f32, tag="sp", name=f"vp{n}{b}1")
                psv2 = sp.tile([128, 1024], f32, tag="sp", name=f"vp{n}{b}2")
                for dt in range(9):
                    _v_chain(n, b, 1, psv1, dt, False)
                    _v_chain(n, b, 2, psv2, dt, False)
                nc.vector.tensor_copy(vaug[n][:, (b * 4 + 1) * VW:(b * 4 + 2) * VW],
                                      psv1[:, :VW])
                nc.vector.tensor_copy(vaug[n][:, (b * 4 + 2) * VW:(b * 4 + 3) * VW],
                                      psv2[:, :VW])
                # pair 3: V chain j=3 & Q chain
                psv3 = sp.tile([128, 1024], f32, tag="sp", name=f"vp{n}{b}3")
                ps_q = sp.tile([128, 1024], f32, tag="sp", name=f"qp{n}{b}")
                for dt in range(9):
                    _v_chain(n, b, 3, psv3, dt, False)
                    if dt < 8:
                        nc.tensor.matmul(ps_q[:DPC, :SW], wq_sb[:, dt, :], xt[:, dt, :],
                                         start=(dt == 0), stop=(dt == 7))
                nc.vector.tensor_copy(vaug[n][:, (b * 4 + 3) * VW:(b * 4 + 4) * VW],
                                      psv3[:, :VW])
                nc.vector.tensor_scalar_add(qT[n][:, b * SW:(b + 1) * SW],
                                            ps_q[:DPC, :SW], bq_sb[:])

            def emit_attn_qs(n, qs):
                kts = [kt for kt in range(NKT) if cls_[qs][kt]]
                exp_tiles = {}
                for kt in kts:
                    a, b = span_[qs][kt]
                    ps = sp.tile([128, 1024], f32, tag="sp", name=f"s{n}{qs}{kt}")
                    for hp in range(HPC):
                        hs = hp * DK
                        nc.tensor.matmul(
                            ps[:KT, hp * SW + a:hp * SW + b],
                            kT[n][hs:hs + DK, kt * KT:(kt + 1) * KT],
                            qT[n][hs:hs + DK, qs * SW + a:qs * SW + b],
                            start=True, stop=True)
                    et = expp.tile([128, 2, SW], bf16, tag="exp", name=f"e{n}{qs}{kt}")
                    nc.scalar.activation(
                        et[:, :, a:b],
                        ps.rearrange("p (h c) -> p h c", h=2)[:, :, a:b], EXP)
                    if cls_[qs][kt] == 2:
                        pi = pidx[qs][kt]
                        for hp in range(HPC):
                            nc.vector.tensor_tensor(
                                et[:, hp, a:b], et[:, hp, a:b],
                                mpat_sb[:, pi, a:b], MUL)
                    exp_tiles[kt] = et
                # AV: the two head chains interleaved so consecutive matmuls
                # alternate PSUM banks (avoids same-bank drain stalls)
                avs = [avp.tile([65, SW], f32, tag="avp", name=f"av{n}{qs}{hp}")
                       for hp in range(HPC)]
                for i, kt in enumerate(kts):
                    a, b = span_[qs][kt]
                    for hp in range(HPC):
                        nc.tensor.matmul(
                            avs[hp][:, a:b],
                            vaug[n][:, kt * VW + hp * 65:kt * VW + (hp + 1) * 65],
                            exp_tiles[kt][:, hp, a:b],
                            start=(i == 0), stop=(i == len(kts) - 1))
                for hp in range(HPC):
                    av = avs[hp]
                    den = nrm.tile([65, SW], bf16, tag="den", name=f"dn{n}{qs}{hp}")
                    nc.vector.tensor_copy(den[64:65, :], av[64:65, :])
                    bc = sp.tile([DK, SW], f32, tag="sp", name=f"bc{n}{qs}{hp}")
                    nc.tensor.matmul(bc[:], ones65[64:65, :], den[64:65, :],
                                     start=True, stop=True)
                    rec = nrm.tile([DK, SW], f32, tag="rec", name=f"rc{n}{qs}{hp}")
                    nc.vector.reciprocal_approx_fast(rec[:], bc[:])
                    nc.vector.tensor_tensor(
                        headT[n][hp][:, qs * SW:(qs + 1) * SW],
                        av[:DK, :], rec[:], MUL)

            def emit_a2a(n):
                for hp in range(HPC):
                    nc.sync.dma_start(
                        a2a_in[n][:, hp * DK:(hp + 1) * DK, :].transpose([1, 0, 2]),
                        headT[n][hp][:, :].rearrange("p (j c) -> p j c", j=NC))
                nc.gpsimd.collective_compute(
                    "AllToAll", mybir.AluOpType.bypass,
                    replica_groups=[list(range(NC))],
                    ins=[a2a_in[n].opt()], outs=[a2a_out[n].opt()])

            def emit_wo(n):
                rhs_t = wos.tile([128, NC, CB], bf16, tag="rhs", name=f"rhs{n}")
                nc.scalar.dma_start(rhs_t[:], a2a_out[n][:, :, :].transpose([1, 0, 2]))
                for mt0 in range(0, 8, 2):
                    pss = [sp.tile([128, 1024], f32, tag="sp", name=f"wp{n}{mt0}{k}")
                           for k in range(2)]
                    for jj in range(8):
                        for k in range(2):
                            mt = mt0 + k
                            nc.tensor.matmul(pss[k][:, :CB],
                                             wo_sb[:, jj, mt * KT:(mt + 1) * KT],
                                             rhs_t[:, jj, :],
                                             start=(jj == 0), stop=(jj == 7))
                    for k in range(2):
                        mt = mt0 + k
                        ob = osb.tile([128, CB], f32, tag="osb", name=f"ob{n}{mt}")
                        nc.vector.tensor_scalar_add(ob[:], pss[k][:, :CB], bo_sb[:, mt, :])
                        nc.scalar.dma_start(out_t[mt * KT:(mt + 1) * KT, n * CB:(n + 1) * CB],
                                            ob[:])

            # ---- pipeline: attention interleaved between projection blocks
            # so PSUM-ring rotation matches data readiness ----
            emit_inputs(0)
            emit_inputs(1)
            emit_proj_block(0, 0)
            emit_proj_block(0, 1)
            emit_attn_qs(0, 0)
            emit_proj_block(0, 2)
            emit_attn_qs(0, 1)
            emit_proj_block(0, 3)
            emit_attn_qs(0, 2)
            emit_proj_block(1, 0)
            emit_attn_qs(0, 3)
            nc.scalar.dma_start(wo_sb[:], wo[:, :, :])
            emit_a2a(0)
            emit_proj_block(1, 1)
            emit_attn_qs(1, 0)
            emit_proj_block(1, 2)
            emit_attn_qs(1, 1)
            emit_proj_block(1, 3)
            emit_attn_qs(1, 2)
            emit_attn_qs(1, 3)
            emit_a2a(1)
            emit_wo(0)
            emit_wo(1)

    nc.compile()
    return nc


def kernel(x, y, mask, Wq, bq, Wk, bk, Wv, bv, Wo, bo, _trace=False):
    x = np.asarray(x, np.float32)
    y = np.asarray(y, np.float32)
    cls_, span_, pidx, pats = _classify_blocks(mask)

    key = (x.shape,
           tuple(tuple(c) for c in cls_),
           tuple(tuple(s) for s in span_),
           tuple(tuple(p) for p in pidx),
           pats.tobytes())
    if key not in _CACHE:
        _CACHE[key] = _build(cls_, span_, pidx, pats.shape[0])
    nc = _CACHE[key]

    fac = np.float32(1.0 / np.sqrt(DK))
    xtb = np.ascontiguousarray(
        np.concatenate([x[n].T for n in range(NB)], axis=1)).astype(BF16)
    ytb = np.ascontiguousarray(
        np.concatenate([y[n].T for n in range(NB)], axis=1)).astype(BF16)
    Wq32 = np.asarray(Wq, np.float32) * fac
    bq32 = np.asarray(bq, np.float32) * fac

    def pmajor(w):
        # [DM, X] -> [128, 8, X] with [p, t, :] = w[t*128+p, :]
        w = np.asarray(w)
        return np.ascontiguousarray(w.reshape(8, 128, w.shape[1]).transpose(1, 0, 2))

    wo_pm = pmajor(np.asarray(Wo, np.float32)).astype(BF16)
    bo_pm = pmajor(np.asarray(bo, np.float32).reshape(DM, 1))
    mpat_t = np.ascontiguousarray(pats.transpose(1, 0, 2))

    in_maps = []
    for c in range(NC):
        d0 = c * DPC
        wv_aug = np.zeros((DM, VW), np.float32)
        bv1 = np.zeros((1, VW), np.float32)
        for hp in range(HPC):
            h = HPC * c + hp
            wv_aug[:, hp * 65:hp * 65 + DK] = np.asarray(Wv, np.float32)[:, h * DK:(h + 1) * DK]
            bv1[0, hp * 65:hp * 65 + DK] = np.asarray(bv, np.float32)[h * DK:(h + 1) * DK]
            bv1[0, hp * 65 + DK] = 1.0
        in_maps.append({
            "xtb": xtb, "ytb": ytb,
            "wq": pmajor(Wq32[:, d0:d0 + DPC]).astype(BF16),
            "wk": pmajor(np.asarray(Wk, np.float32)[:, d0:d0 + DPC]).astype(BF16),
            "wv": pmajor(wv_aug).astype(BF16),
            "wo": wo_pm,
            "bq": bq32[d0:d0 + DPC].reshape(DPC, 1),
            "bk": np.asarray(bk, np.float32)[d0:d0 + DPC].reshape(DPC, 1),
            "bv1": bv1.astype(BF16),
            "bo": bo_pm,
            "mpat": mpat_t,
        })

    res = run_bass_kernel_spmd(nc, in_maps, core_ids=list(range(NC)), trace=_trace)
    out = np.empty((NB, L, DM), np.float32)
    for c in range(NC):
        for n in range(NB):
            out[n, c * CB:(c + 1) * CB, :] = res.results[c]["out_t"][:, n * CB:(n + 1) * CB].T
    if _trace:
        kernel.last_results = res
    return out
